# revision 22
# baseline (speedup 1.0000x reference)
"""Causal self-attention Trainium2 kernel (B=2, T=2048, C=1024, H=16, D=64).

Sharding: 8 cores = data-parallel on B (2) x tensor-parallel on heads (16/4=4
heads per core). Column-parallel Wqkv, row-parallel Wproj; the row-parallel
partial outputs are summed on the host.

v8 design (instruction-count reduction + cross-rep pipelining + fused
diagonal):
  - bf16 datapath, fp32 PSUM accumulation, 256-token attention slices,
    feature-major host-transposed x, flash-style S^T attention with the
    65th-row-of-ones denominator trick and PE row-group (tile_position)
    pairing of the two heads of a 128-partition pair.
  - q/k projections per slice-PAIR with 512-column streams; 512-column
    output projection streams; DVE triangular-mask multiply for the causal
    diagonal; ones/mask initialized once outside the rep loop.
  - all SBUF/PSUM pools and the large persistent tiles live at program
    scope: double-buffered by rep parity (wq/kT/yT/wp/vaug) or ring-shared
    (x, qts, expS, psum pools). Consecutive reps therefore pipeline — the
    next rep's weight/x DMAs and qkv matmuls overlap the previous rep's
    attention tail instead of serializing on pool teardown.
"""

import numpy as np

import concourse.bacc as bacc
import concourse.mybir as mybir
import concourse.tile as tile
from concourse.bass_utils import run_bass_kernel_spmd

B, T, C, H, D = 2, 2048, 1024, 16, 64
NCORES = 8
HPC = H // (NCORES // B)  # 4 heads per core
DSH = HPC * D             # 256 head-dims per core
P = 128
TS = 256                  # q/t slice width (attention granularity)
PW = 512                  # slice-pair width (qkv/proj stream width)
NTS = T // TS             # 8 slices
NT = T // P               # 16 k-tiles
CS = C // P               # 8 contraction subtiles
TPS = TS // P             # 2 t-tiles per slice

f32 = mybir.dt.float32
bf16 = mybir.dt.bfloat16
FP = mybir.ActivationFunctionType
NPBF16 = mybir.dt.np(bf16)


def build_program(reps=1, use_bias=False):
    nc = bacc.Bacc("TRN2", debug=False, num_devices=NCORES)
    x_d = nc.dram_tensor("x", [P, CS, T], bf16, kind="ExternalInput").ap()
    wqkv_d = nc.dram_tensor("wqkv", [6, P, CS, P], bf16, kind="ExternalInput").ap()
    bqkv_d = nc.dram_tensor("bqkv", [3 * DSH], f32, kind="ExternalInput").ap()
    wproj_d = nc.dram_tensor("wproj", [DSH, C], bf16, kind="ExternalInput").ap()
    out_d = nc.dram_tensor("out", [T, C], bf16, kind="ExternalOutput").ap()

    with tile.TileContext(nc) as tc:
        from contextlib import ExitStack

        ctx = ExitStack()
        with ctx:
            ep = ctx.enter_context
            gpool = ep(tc.tile_pool(name="globals", bufs=1))
            R = {}
            # rep-parity double buffers
            R["wq_sb"] = [gpool.tile([P, 6, CS, P], bf16, name=f"wq{i}") for i in range(2)]
            R["kT_sb"] = [gpool.tile([P, 2, T], bf16, name=f"kT{i}") for i in range(2)]
            R["yT"] = [gpool.tile([P, 2, T], bf16, name=f"yT{i}") for i in range(2)]
            R["wp_sb"] = [gpool.tile([P, 2, C], bf16, name=f"wp{i}") for i in range(2)]
            R["vaug"] = [gpool.tile([P, NT, HPC, 65], bf16, name=f"va{i}") for i in range(2)]
            R["mask_sb"] = gpool.tile([P, P], bf16, name="mask")
            R["mask2"] = gpool.tile([P, 2, P], bf16, name="mask2")
            # bias tiles (unused when use_bias=False)
            R["bias_col"] = gpool.tile([P, 4], f32, name="bias_col")
            R["bias_v"] = gpool.tile([P, DSH], f32, name="bias_v")
            R["bias_v1"] = gpool.tile([1, DSH], f32, name="bias_v1")
            # shared pools (ring-rotated across reps)
            R["xsb"] = ep(tc.tile_pool(name="xsb", bufs=4))
            R["qts"] = ep(tc.tile_pool(name="qts", bufs=2))
            R["expS"] = ep(tc.tile_pool(name="expS", bufs=6))
            R["bc"] = ep(tc.tile_pool(name="bc", bufs=6))
            R["outsb"] = ep(tc.tile_pool(name="outsb", bufs=4))
            R["pmm"] = ep(tc.tile_pool(name="pmm", bufs=2, space="PSUM"))
            R["ps"] = ep(tc.tile_pool(name="ps", bufs=2, space="PSUM"))
            R["py"] = ep(tc.tile_pool(name="py", bufs=2, space="PSUM"))

            # norm/proj deferral state carried across reps: the last slice's
            # softmax normalization and output projections of rep n run as
            # PE fillers inside rep n+1 instead of serializing at the tail
            R["pending"] = None
            R["proj_fill"] = []
            for va in R["vaug"]:
                nc.vector.memset(va[:, :, :, 64], 1.0)
            nc.vector.memset(R["mask_sb"], 1.0)
            nc.gpsimd.affine_select(
                out=R["mask_sb"],
                in_=R["mask_sb"],
                compare_op=mybir.AluOpType.is_ge,
                fill=0.0,
                base=0,
                channel_multiplier=-1,
                pattern=[[1, P]],
            )
            nc.vector.memset(R["mask2"], 1.0)
            nc.gpsimd.affine_select(
                out=R["mask2"],
                in_=R["mask2"],
                compare_op=mybir.AluOpType.is_ge,
                fill=0.0,
                base=0,
                channel_multiplier=-1,
                pattern=[[0, 2], [1, P]],
            )
            for rep in range(reps):
                kernel_body(tc, rep, R, x_d, wqkv_d, bqkv_d, wproj_d, out_d,
                            use_bias)
            flush_tail(tc, R, out_d)
    nc.compile()
    return nc


def flush_tail(tc, R, out_d):
    """Emit the final rep's deferred norm + projections."""
    nc = tc.nc
    if R["pending"] is not None:
        emit_norm_g(tc, R, R["pending"])
        f_si = R["pending"][0]
        yT_ref, wp_ref = R["pending"][4], R["pending"][5]
        for qq in range(TPS):
            R["proj_fill"].append((f_si, qq, yT_ref, wp_ref))
        R["pending"] = None
    for f_si, qq, yT_ref, wp_ref in R["proj_fill"]:
        ob_t = R["outsb"].tile([P, C], bf16, name="ob_t")
        emit_proj_g(tc, R, f_si, qq, yT_ref, wp_ref, ob_t, out_d)
    R["proj_fill"] = []


def emit_norm_g(tc, R, p):
    nc = tc.nc
    f_si, f_qsl, f_py0, f_py1, yT_ref, wp_ref = p
    for hp, py_t in ((0, f_py0), (1, f_py1)):
        rc_t = R["bc"].tile([1, 2, TS], f32, name="rc_t", tag="rc")
        nc.vector.reciprocal(rc_t, py_t[64:65, :, :])
        bc_t = R["bc"].tile([64, 2, TS], f32, name="bc_t", tag="bc")
        nc.gpsimd.partition_broadcast(bc_t, rc_t, channels=64)
        for hh in range(2):
            hb = hh * 64
            nc.vector.tensor_mul(
                yT_ref[hb : hb + 64, hp, f_qsl],
                py_t[0:64, hh, :],
                bc_t[:, hh, :],
            )


def emit_proj_g(tc, R, f_si, qq, yT_ref, wp_ref, ob_t, out_d):
    nc = tc.nc
    qt = f_si * TPS + qq
    for cc in range(2):
        po_t = R["pmm"].tile([P, PW], f32, name="po_t", tag="pmm")
        for chp in range(2):
            nc.tensor.matmul(
                po_t,
                lhsT=yT_ref[:, chp, qt * P : (qt + 1) * P],
                rhs=wp_ref[:, chp, cc * PW : (cc + 1) * PW],
                start=(chp == 0),
                stop=(chp == 1),
            )
        nc.vector.tensor_copy(ob_t[:, cc * PW : (cc + 1) * PW], po_t)
    nc.sync.dma_start(out_d[qt * P : (qt + 1) * P, :], ob_t)


def kernel_body(tc, rep, R, x_d, wqkv_d, bqkv_d, wproj_d, out_d,
                use_bias=False):
    nc = tc.nc
    par_ = rep % 2
    wq_sb = R["wq_sb"][par_]
    kT_sb = R["kT_sb"][par_]
    yT = R["yT"][par_]
    wp_sb = R["wp_sb"][par_]
    vaug = R["vaug"][par_]
    mask_sb = R["mask_sb"]
    mask2 = R["mask2"]
    bias_col, bias_v, bias_v1 = R["bias_col"], R["bias_v"], R["bias_v1"]
    xsb_pool, qts_pool = R["xsb"], R["qts"]
    expS_pool, bc_pool, outsb_pool = R["expS"], R["bc"], R["outsb"]
    pmm_pool, ps_pool, py_pool = R["pmm"], R["ps"], R["py"]
    wq_src = wqkv_d.rearrange("ch p cs f -> p ch cs f")

    # paired q/k emission: 512-column streams over a slice-pair's xT
    def emit_qk(ch, pj, qTs, xTs):
        pq = pmm_pool.tile([P, PW], f32, name="pq", tag="pmm")
        for cs in range(CS):
            nc.tensor.matmul(
                pq,
                lhsT=wq_sb[:, ch, cs, :],
                rhs=xTs[:, cs, :],
                start=(cs == 0),
                stop=(cs == CS - 1),
            )
        if ch < 2:
            dst = qTs[:, ch, :]
        else:
            dst = kT_sb[:, ch - 2, pj * PW : (pj + 1) * PW]
        if use_bias:
            nc.vector.tensor_scalar_add(dst, pq, bias_col[:, ch : ch + 1])
        else:
            nc.vector.tensor_copy(dst, pq)

    def emit_v(si, a, xTs, xoff):
        kt = TPS * si + a
        pv = pmm_pool.tile([P, DSH], f32, name="pv", tag="pmm")
        for cs in range(CS):
            nc.tensor.matmul(
                pv,
                lhsT=xTs[:, cs, xoff + a * P : xoff + (a + 1) * P],
                rhs=wq_sb[:, 4:6, cs, :],
                start=(cs == 0),
                stop=(cs == CS - 1),
            )
        dst = vaug[:, kt, :, 0:64]
        src = pv.rearrange("p (h d) -> p h d", h=HPC)
        if use_bias:
            nc.vector.tensor_add(
                dst, src, bias_v.rearrange("p (h d) -> p h d", h=HPC)
            )
        else:
            nc.vector.tensor_copy(dst, src)

    # Deferred PV per head-pair: emit S+exp for a tile (or hist pair), then
    # flush the pending PVs of the previous tile, keeping PE ahead of ACT.
    pend_pv = [None, None]

    def emit_pv(hp):
        if pend_pv[hp] is None:
            return
        si, py_t, entries = pend_pv[hp]
        pend_pv[hp] = None
        n_k = TPS * (si + 1)
        for kt, qoff, rhss in entries:
            for hh in range(2):
                # the two heads share one PSUM bank: only the first matmul
                # of the group clears it, only the last stops it
                nc.tensor.matmul(
                    py_t[:65, hh, qoff:TS],
                    lhsT=vaug[:, kt, 2 * hp + hh, :],
                    rhs=rhss[hh],
                    start=(kt == 0 and hh == 0),
                    stop=(kt == n_k - 1 and hh == 1),
                )

    def emit_s_pair(si, hp, kt0, qTs, qoff0, py01):
        # two full-width history k-tiles fused into one exp instruction
        ps_t = ps_pool.tile([P, 2, 2, TS], f32, name="ps_t", tag="ps")
        ex_t = expS_pool.tile([P, 2, 2, TS], bf16, name="ex_t")
        for par in range(2):
            for hh in range(2):
                hb = hh * 64
                nc.tensor.matmul(
                    ps_t[:, hh, par, :],
                    lhsT=kT_sb[hb : hb + 64, hp, (kt0 + par) * P : (kt0 + par + 1) * P],
                    rhs=qTs[hb : hb + 64, hp, qoff0 : qoff0 + TS],
                    start=True,
                    stop=True,
                    tile_position=(hb, 0),
                )
        nc.scalar.activation(ex_t, ps_t, FP.Exp, scale=0.125)
        emit_pv(hp)
        pend_pv[hp] = (
            si,
            py01,
            [
                (kt0, 0, [ex_t[:, 0, 0, :], ex_t[:, 1, 0, :]]),
                (kt0 + 1, 0, [ex_t[:, 0, 1, :], ex_t[:, 1, 1, :]]),
            ],
        )

    def emit_s_diag(si, hp, qTs, qoff0, py01):
        # the slice's two diagonal k-tiles fused: one exp per head-pair, one
        # two-head mask multiply per k-tile. ps[:, hh, 1, 0:P] is never
        # written; its exp output is masked garbage that no PV reads.
        kt0 = TPS * si
        ps_t = ps_pool.tile([P, 2, 2, TS], f32, name="ps_t", tag="ps")
        ex_t = expS_pool.tile([P, 2, 2, TS], bf16, name="ex_t")
        for kd in range(2):
            qoff = kd * P
            for hh in range(2):
                hb = hh * 64
                nc.tensor.matmul(
                    ps_t[:, hh, kd, qoff:TS],
                    lhsT=kT_sb[hb : hb + 64, hp, (kt0 + kd) * P : (kt0 + kd + 1) * P],
                    rhs=qTs[hb : hb + 64, hp, qoff0 + qoff : qoff0 + TS],
                    start=True,
                    stop=True,
                    tile_position=(hb, 0),
                )
        nc.scalar.activation(ex_t, ps_t, FP.Exp, scale=0.125)
        nc.vector.tensor_mul(ex_t[:, :, 0, 0:P], ex_t[:, :, 0, 0:P], mask2)
        nc.vector.tensor_mul(ex_t[:, :, 1, P:TS], ex_t[:, :, 1, P:TS], mask2)
        emit_pv(hp)
        pend_pv[hp] = (
            si,
            py01,
            [
                (kt0, 0, [ex_t[:, 0, 0, :], ex_t[:, 1, 0, :]]),
                (kt0 + 1, P, [ex_t[:, 0, 1, P:TS], ex_t[:, 1, 1, P:TS]]),
            ],
        )

    def py_pair():
        return py_pool.tile([P, 2, TS], f32, name="py", tag="py")

    xtiles = {}

    def x_load(pj, split=False):
        t_ = xsb_pool.tile([P, CS, PW], bf16, name="x_sb")
        if split:
            h = CS // 2
            nc.sync.dma_start(t_[:, 0:h], x_d[:, 0:h, pj * PW : (pj + 1) * PW])
            nc.sync.dma_start(t_[:, h:CS], x_d[:, h:CS, pj * PW : (pj + 1) * PW])
        else:
            nc.sync.dma_start(t_, x_d[:, :, pj * PW : (pj + 1) * PW])
        xtiles[pj] = t_

    nc.sync.dma_start(wq_sb[:, 0], wq_src[:, 0])
    x_load(0, split=True)
    for ch in range(1, 4):
        nc.sync.dma_start(wq_sb[:, ch], wq_src[:, ch])
    nc.sync.dma_start(wq_sb[:, 4:6], wq_src[:, 4:6])
    x_load(1)
    nc.sync.dma_start(wp_sb, wproj_d.rearrange("(ch p) f -> p ch f", p=P))
    for pj in range(2, NTS // 2):
        x_load(pj)
    qTs = None
    for si in range(NTS):
        pj, spar = divmod(si, 2)
        xTs = xtiles[pj] if spar == 0 else xtiles.pop(pj)
        xoff = spar * TS
        if spar == 0:
            qTs = qts_pool.tile([P, 2, PW], bf16, name="qTs")
            if si == 0 and use_bias:
                nc.sync.dma_start(
                    bias_col,
                    bqkv_d[0 : 4 * P].rearrange("(ch p) -> p ch", p=P),
                )
                nc.sync.dma_start(
                    bias_v1, bqkv_d[2 * DSH : 3 * DSH].rearrange("f -> 1 f")
                )
                nc.gpsimd.partition_broadcast(bias_v, bias_v1, channels=P)
            emit_qk(0, pj, qTs, xTs)
            emit_qk(1, pj, qTs, xTs)
        py01s = [py_pair(), py_pair()]
        if R["pending"] is not None:
            emit_norm_g(tc, R, R["pending"])

        # filler units: previous slices' projections + this pair's K (even
        # slice only) and this slice's V
        if R["pending"] is not None:
            p = R["pending"]
            for qq in range(TPS):
                R["proj_fill"].append((p[0], qq, p[4], p[5]))
        R["pending"] = None
        if spar == 0:
            kv_fill = [("qk", 2), ("qk", 3), ("v", 0), ("v", 1)]
        else:
            kv_fill = [("v", 0), ("v", 1)]

        # hp-blocked order: with 2 ps slots this gives depth-2 run-ahead
        # within each head-pair's S->exp chain. History tiles come in fused
        # kt-pairs (2si is always even).
        hist_units = [
            (hp, kt0) for hp in range(2) for kt0 in range(0, TPS * si, 2)
        ]

        def pop_filler(allow_kv=True, allow_proj=True):
            if allow_kv and kv_fill:
                f = kv_fill.pop(0)
                if f[0] == "qk":
                    emit_qk(f[1], pj, qTs, xTs)
                else:
                    emit_v(si, f[1], xTs, xoff)
                return True
            if allow_proj and R["proj_fill"]:
                f_si, qq, yT_ref, wp_ref = R["proj_fill"].pop(0)
                ob_t = outsb_pool.tile([P, C], bf16, name="ob_t")
                emit_proj_g(tc, R, f_si, qq, yT_ref, wp_ref, ob_t, out_d)
                return True
            return False

        for i, (hp, kt0) in enumerate(hist_units):
            emit_s_pair(si, hp, kt0, qTs, xoff, py01s[hp])
            pop_filler()
        # K/V for this slice must be complete before the diagonal
        while pop_filler(allow_proj=False):
            pass
        for hp in range(2):
            emit_s_diag(si, hp, qTs, xoff, py01s[hp])
            pop_filler(allow_kv=False)
        emit_pv(0)
        emit_pv(1)
        R["pending"] = (
            si, slice(si * TS, (si + 1) * TS), py01s[0], py01s[1], yT, wp_sb
        )


_NC_CACHE = {}


def get_program(use_bias=False):
    key = ("nc", use_bias)
    if key not in _NC_CACHE:
        _NC_CACHE[key] = build_program(use_bias=use_bias)
    return _NC_CACHE[key]


def shard_inputs(x, w_qkv, b_qkv, w_proj):
    """Per-core input dicts: core c -> batch c//4, head-group c%4."""
    x = np.asarray(x, dtype=np.float32).astype(NPBF16)
    w_qkv = np.asarray(w_qkv, dtype=np.float32).astype(NPBF16)
    b_qkv = np.asarray(b_qkv, dtype=np.float32)
    w_proj = np.asarray(w_proj, dtype=np.float32).astype(NPBF16)
    in_maps = []
    for c in range(NCORES):
        b, g = divmod(c, NCORES // B)
        cols = []
        for r_ in range(3):  # q, k, v regions
            lo = r_ * C + g * DSH
            cols.append(np.arange(lo, lo + DSH))
        cols = np.concatenate(cols)
        wq = w_qkv[:, cols]  # [C, 3*DSH]
        # chunk-major device layout: [ch, p, cs, fo]
        wq_dev = np.ascontiguousarray(
            wq.reshape(CS, P, 6, P).transpose(2, 1, 0, 3)
        )
        in_maps.append(
            {
                "x": np.ascontiguousarray(
                    x[b].reshape(T, CS, P).transpose(2, 1, 0)
                ),
                "wqkv": wq_dev,
                "bqkv": np.ascontiguousarray(b_qkv[cols]),
                "wproj": np.ascontiguousarray(w_proj[g * DSH : (g + 1) * DSH, :]),
            }
        )
    return in_maps


def kernel(x, w_qkv, b_qkv, w_proj, b_proj, _trace=False):
    use_bias = bool(np.any(np.asarray(b_qkv)))
    nc = get_program(use_bias)
    in_maps = shard_inputs(x, w_qkv, b_qkv, w_proj)
    res = run_bass_kernel_spmd(nc, in_maps, core_ids=list(range(NCORES)), trace=_trace)
    out = np.zeros((B, T, C), dtype=np.float32)
    for c in range(NCORES):
        out[c // (NCORES // B)] += res.results[c]["out"].astype(np.float32)
    out += np.asarray(b_proj, dtype=np.float32)[None, None, :]
    if _trace:
        kernel._last_results = res
    return out


# revision 24
# speedup vs baseline: 1.0517x; 1.0517x over previous
"""Causal self-attention Trainium2 kernel (B=2, T=2048, C=1024, H=16, D=64).

Sharding: 8 cores = data-parallel on B (2) x tensor-parallel on heads (16/4=4
heads per core). Column-parallel Wqkv, row-parallel Wproj; the row-parallel
partial outputs are summed on the host.

v8 design (instruction-count reduction + cross-rep pipelining + fused
diagonal):
  - bf16 datapath, fp32 PSUM accumulation, 256-token attention slices,
    feature-major host-transposed x, flash-style S^T attention with the
    65th-row-of-ones denominator trick and PE row-group (tile_position)
    pairing of the two heads of a 128-partition pair.
  - q/k projections per slice-PAIR with 512-column streams; 512-column
    output projection streams; DVE triangular-mask multiply for the causal
    diagonal; ones/mask initialized once outside the rep loop.
  - all SBUF/PSUM pools and the large persistent tiles live at program
    scope: double-buffered by rep parity (wq/kT/yT/wp/vaug) or ring-shared
    (x, qts, expS, psum pools). Consecutive reps therefore pipeline — the
    next rep's weight/x DMAs and qkv matmuls overlap the previous rep's
    attention tail instead of serializing on pool teardown.
"""

import numpy as np

import concourse.bacc as bacc
import concourse.mybir as mybir
import concourse.tile as tile
from concourse.bass_utils import run_bass_kernel_spmd

B, T, C, H, D = 2, 2048, 1024, 16, 64
NCORES = 8
HPC = H // (NCORES // B)  # 4 heads per core
DSH = HPC * D             # 256 head-dims per core
P = 128
TS = 256                  # q/t slice width (attention granularity)
PW = 512                  # slice-pair width (qkv/proj stream width)
NTS = T // TS             # 8 slices
NT = T // P               # 16 k-tiles
CS = C // P               # 8 contraction subtiles
TPS = TS // P             # 2 t-tiles per slice

f32 = mybir.dt.float32
bf16 = mybir.dt.bfloat16
FP = mybir.ActivationFunctionType
NPBF16 = mybir.dt.np(bf16)


def build_program(reps=1, use_bias=False):
    nc = bacc.Bacc("TRN2", debug=False, num_devices=NCORES)
    x_d = nc.dram_tensor("x", [P, CS, T], bf16, kind="ExternalInput").ap()
    wqkv_d = nc.dram_tensor("wqkv", [6, P, CS, P], bf16, kind="ExternalInput").ap()
    bqkv_d = nc.dram_tensor("bqkv", [3 * DSH], f32, kind="ExternalInput").ap()
    wproj_d = nc.dram_tensor("wproj", [DSH, C], bf16, kind="ExternalInput").ap()
    out_d = nc.dram_tensor("out", [T, C], bf16, kind="ExternalOutput").ap()

    with tile.TileContext(nc) as tc:
        from contextlib import ExitStack

        ctx = ExitStack()
        with ctx:
            ep = ctx.enter_context
            gpool = ep(tc.tile_pool(name="globals", bufs=1))
            R = {}
            # rep-parity double buffers
            R["wq_sb"] = [gpool.tile([P, 6, CS, P], bf16, name=f"wq{i}") for i in range(2)]
            R["kT_sb"] = [gpool.tile([P, 2, T], bf16, name=f"kT{i}") for i in range(2)]
            R["yT"] = [gpool.tile([P, 2, T], bf16, name=f"yT{i}") for i in range(2)]
            R["wp_sb"] = [gpool.tile([P, 2, C], bf16, name=f"wp{i}") for i in range(2)]
            R["vaug"] = [gpool.tile([P, NT, HPC, 65], bf16, name=f"va{i}") for i in range(2)]
            R["mask_sb"] = gpool.tile([P, P], bf16, name="mask")
            R["mask2"] = gpool.tile([P, 2, P], bf16, name="mask2")
            # bias tiles (unused when use_bias=False)
            R["bias_col"] = gpool.tile([P, 4], f32, name="bias_col")
            R["bias_v"] = gpool.tile([P, DSH], f32, name="bias_v")
            R["bias_v1"] = gpool.tile([1, DSH], f32, name="bias_v1")
            # shared pools (ring-rotated across reps)
            R["xsb"] = ep(tc.tile_pool(name="xsb", bufs=4))
            R["qts"] = ep(tc.tile_pool(name="qts", bufs=2))
            R["expS"] = ep(tc.tile_pool(name="expS", bufs=6))
            R["bc"] = ep(tc.tile_pool(name="bc", bufs=8))
            R["outsb"] = ep(tc.tile_pool(name="outsb", bufs=4))
            R["pmm"] = ep(tc.tile_pool(name="pmm", bufs=2, space="PSUM"))
            R["ps"] = ep(tc.tile_pool(name="ps", bufs=2, space="PSUM"))
            R["py"] = ep(tc.tile_pool(name="py", bufs=2, space="PSUM"))

            # norm/proj deferral state carried across reps: the last slice's
            # softmax normalization and output projections of rep n run as
            # PE fillers inside rep n+1 instead of serializing at the tail
            R["pending"] = None
            R["proj_fill"] = []
            for va in R["vaug"]:
                nc.vector.memset(va[:, :, :, 64], 1.0)
            nc.vector.memset(R["mask_sb"], 1.0)
            nc.gpsimd.affine_select(
                out=R["mask_sb"],
                in_=R["mask_sb"],
                compare_op=mybir.AluOpType.is_ge,
                fill=0.0,
                base=0,
                channel_multiplier=-1,
                pattern=[[1, P]],
            )
            nc.vector.memset(R["mask2"], 1.0)
            nc.gpsimd.affine_select(
                out=R["mask2"],
                in_=R["mask2"],
                compare_op=mybir.AluOpType.is_ge,
                fill=0.0,
                base=0,
                channel_multiplier=-1,
                pattern=[[0, 2], [1, P]],
            )
            for rep in range(reps):
                kernel_body(tc, rep, R, x_d, wqkv_d, bqkv_d, wproj_d, out_d,
                            use_bias)
            flush_tail(tc, R, out_d)
    nc.compile()
    return nc


def flush_tail(tc, R, out_d):
    """Emit the final rep's deferred norm + projections."""
    nc = tc.nc
    if R["pending"] is not None:
        emit_norm_g(tc, R, R["pending"])
        f_si = R["pending"][0]
        yT_ref, wp_ref = R["pending"][4], R["pending"][5]
        for qq in range(TPS):
            R["proj_fill"].append((f_si, qq, yT_ref, wp_ref))
        R["pending"] = None
    for f_si, qq, yT_ref, wp_ref in R["proj_fill"]:
        ob_t = R["outsb"].tile([P, C], bf16, name="ob_t")
        emit_proj_g(tc, R, f_si, qq, yT_ref, wp_ref, ob_t, out_d)
    R["proj_fill"] = []


def emit_norm_g(tc, R, p):
    nc = tc.nc
    f_si, f_qsl, f_py0, f_py1, yT_ref, wp_ref = p
    for hp, py_t in ((0, f_py0), (1, f_py1)):
        # stage the whole accumulator (64 d rows + denominator row) into
        # SBUF in ONE copy: the PSUM slot frees after ~0.7us instead of
        # being held through the whole recip->broadcast->mul chain, so the
        # next slice's PV never waits on normalization.
        st_t = R["bc"].tile([65, 2, TS], f32, name="st_t", tag="st")
        nc.vector.tensor_copy(st_t, py_t[0:65, :, :])
        rc_t = R["bc"].tile([1, 2, TS], f32, name="rc_t", tag="rc")
        nc.vector.reciprocal(rc_t, st_t[64:65, :, :])
        bc_t = R["bc"].tile([64, 2, TS], f32, name="bc_t", tag="bc")
        nc.gpsimd.partition_broadcast(bc_t, rc_t, channels=64)
        for hh in range(2):
            hb = hh * 64
            nc.vector.tensor_mul(
                yT_ref[hb : hb + 64, hp, f_qsl],
                st_t[0:64, hh, :],
                bc_t[:, hh, :],
            )


def emit_proj_g(tc, R, f_si, qq, yT_ref, wp_ref, ob_t, out_d):
    nc = tc.nc
    qt = f_si * TPS + qq
    for cc in range(2):
        po_t = R["pmm"].tile([P, PW], f32, name="po_t", tag="pmm")
        for chp in range(2):
            nc.tensor.matmul(
                po_t,
                lhsT=yT_ref[:, chp, qt * P : (qt + 1) * P],
                rhs=wp_ref[:, chp, cc * PW : (cc + 1) * PW],
                start=(chp == 0),
                stop=(chp == 1),
            )
        nc.vector.tensor_copy(ob_t[:, cc * PW : (cc + 1) * PW], po_t)
    nc.sync.dma_start(out_d[qt * P : (qt + 1) * P, :], ob_t)


def kernel_body(tc, rep, R, x_d, wqkv_d, bqkv_d, wproj_d, out_d,
                use_bias=False):
    nc = tc.nc
    par_ = rep % 2
    wq_sb = R["wq_sb"][par_]
    kT_sb = R["kT_sb"][par_]
    yT = R["yT"][par_]
    wp_sb = R["wp_sb"][par_]
    vaug = R["vaug"][par_]
    mask_sb = R["mask_sb"]
    mask2 = R["mask2"]
    bias_col, bias_v, bias_v1 = R["bias_col"], R["bias_v"], R["bias_v1"]
    xsb_pool, qts_pool = R["xsb"], R["qts"]
    expS_pool, bc_pool, outsb_pool = R["expS"], R["bc"], R["outsb"]
    pmm_pool, ps_pool, py_pool = R["pmm"], R["ps"], R["py"]
    wq_src = wqkv_d.rearrange("ch p cs f -> p ch cs f")

    # paired q/k emission: 512-column streams over a slice-pair's xT
    def emit_qk(ch, pj, qTs, xTs):
        pq = pmm_pool.tile([P, PW], f32, name="pq", tag="pmm")
        for cs in range(CS):
            nc.tensor.matmul(
                pq,
                lhsT=wq_sb[:, ch, cs, :],
                rhs=xTs[:, cs, :],
                start=(cs == 0),
                stop=(cs == CS - 1),
            )
        if ch < 2:
            dst = qTs[:, ch, :]
        else:
            dst = kT_sb[:, ch - 2, pj * PW : (pj + 1) * PW]
        if use_bias:
            nc.vector.tensor_scalar_add(dst, pq, bias_col[:, ch : ch + 1])
        else:
            nc.vector.tensor_copy(dst, pq)

    def emit_v(si, a, xTs, xoff):
        kt = TPS * si + a
        pv = pmm_pool.tile([P, DSH], f32, name="pv", tag="pmm")
        for cs in range(CS):
            nc.tensor.matmul(
                pv,
                lhsT=xTs[:, cs, xoff + a * P : xoff + (a + 1) * P],
                rhs=wq_sb[:, 4:6, cs, :],
                start=(cs == 0),
                stop=(cs == CS - 1),
            )
        dst = vaug[:, kt, :, 0:64]
        src = pv.rearrange("p (h d) -> p h d", h=HPC)
        if use_bias:
            nc.vector.tensor_add(
                dst, src, bias_v.rearrange("p (h d) -> p h d", h=HPC)
            )
        else:
            nc.vector.tensor_copy(dst, src)

    # Deferred PV per head-pair: emit S+exp for a tile (or hist pair), then
    # flush the pending PVs of the previous tile, keeping PE ahead of ACT.
    pend_pv = [None, None]

    def emit_pv(hp):
        if pend_pv[hp] is None:
            return
        si, py_t, entries = pend_pv[hp]
        pend_pv[hp] = None
        n_k = TPS * (si + 1)
        for kt, qoff, rhss in entries:
            for hh in range(2):
                # the two heads share one PSUM bank: only the first matmul
                # of the group clears it, only the last stops it
                nc.tensor.matmul(
                    py_t[:65, hh, qoff:TS],
                    lhsT=vaug[:, kt, 2 * hp + hh, :],
                    rhs=rhss[hh],
                    start=(kt == 0 and hh == 0),
                    stop=(kt == n_k - 1 and hh == 1),
                )

    def emit_s_pair(si, hp, kt0, qTs, qoff0, py01):
        # two full-width history k-tiles fused into one exp instruction
        ps_t = ps_pool.tile([P, 2, 2, TS], f32, name="ps_t", tag="ps")
        ex_t = expS_pool.tile([P, 2, 2, TS], bf16, name="ex_t")
        for par in range(2):
            for hh in range(2):
                hb = hh * 64
                nc.tensor.matmul(
                    ps_t[:, hh, par, :],
                    lhsT=kT_sb[hb : hb + 64, hp, (kt0 + par) * P : (kt0 + par + 1) * P],
                    rhs=qTs[hb : hb + 64, hp, qoff0 : qoff0 + TS],
                    start=True,
                    stop=True,
                    tile_position=(hb, 0),
                )
        nc.scalar.activation(ex_t, ps_t, FP.Exp, scale=0.125)
        emit_pv(hp)
        pend_pv[hp] = (
            si,
            py01,
            [
                (kt0, 0, [ex_t[:, 0, 0, :], ex_t[:, 1, 0, :]]),
                (kt0 + 1, 0, [ex_t[:, 0, 1, :], ex_t[:, 1, 1, :]]),
            ],
        )

    def emit_s_diag(si, hp, qTs, qoff0, py01):
        # the slice's two diagonal k-tiles fused: one exp per head-pair, one
        # two-head mask multiply per k-tile. ps[:, hh, 1, 0:P] is never
        # written; its exp output is masked garbage that no PV reads.
        kt0 = TPS * si
        ps_t = ps_pool.tile([P, 2, 2, TS], f32, name="ps_t", tag="ps")
        ex_t = expS_pool.tile([P, 2, 2, TS], bf16, name="ex_t")
        for kd in range(2):
            qoff = kd * P
            for hh in range(2):
                hb = hh * 64
                nc.tensor.matmul(
                    ps_t[:, hh, kd, qoff:TS],
                    lhsT=kT_sb[hb : hb + 64, hp, (kt0 + kd) * P : (kt0 + kd + 1) * P],
                    rhs=qTs[hb : hb + 64, hp, qoff0 + qoff : qoff0 + TS],
                    start=True,
                    stop=True,
                    tile_position=(hb, 0),
                )
        nc.scalar.activation(ex_t, ps_t, FP.Exp, scale=0.125)
        nc.vector.tensor_mul(ex_t[:, :, 0, 0:P], ex_t[:, :, 0, 0:P], mask2)
        nc.vector.tensor_mul(ex_t[:, :, 1, P:TS], ex_t[:, :, 1, P:TS], mask2)
        emit_pv(hp)
        pend_pv[hp] = (
            si,
            py01,
            [
                (kt0, 0, [ex_t[:, 0, 0, :], ex_t[:, 1, 0, :]]),
                (kt0 + 1, P, [ex_t[:, 0, 1, P:TS], ex_t[:, 1, 1, P:TS]]),
            ],
        )

    def py_pair():
        return py_pool.tile([P, 2, TS], f32, name="py", tag="py")

    xtiles = {}

    def x_load(pj, split=False):
        t_ = xsb_pool.tile([P, CS, PW], bf16, name="x_sb")
        if split:
            h = CS // 2
            nc.sync.dma_start(t_[:, 0:h], x_d[:, 0:h, pj * PW : (pj + 1) * PW])
            nc.sync.dma_start(t_[:, h:CS], x_d[:, h:CS, pj * PW : (pj + 1) * PW])
        else:
            nc.sync.dma_start(t_, x_d[:, :, pj * PW : (pj + 1) * PW])
        xtiles[pj] = t_

    nc.sync.dma_start(wq_sb[:, 0], wq_src[:, 0])
    x_load(0, split=True)
    for ch in range(1, 4):
        nc.sync.dma_start(wq_sb[:, ch], wq_src[:, ch])
    nc.sync.dma_start(wq_sb[:, 4:6], wq_src[:, 4:6])
    x_load(1)
    nc.sync.dma_start(wp_sb, wproj_d.rearrange("(ch p) f -> p ch f", p=P))
    for pj in range(2, NTS // 2):
        x_load(pj)
    qTs = None
    for si in range(NTS):
        pj, spar = divmod(si, 2)
        xTs = xtiles[pj] if spar == 0 else xtiles.pop(pj)
        xoff = spar * TS
        if spar == 0:
            qTs = qts_pool.tile([P, 2, PW], bf16, name="qTs")
            if si == 0 and use_bias:
                nc.sync.dma_start(
                    bias_col,
                    bqkv_d[0 : 4 * P].rearrange("(ch p) -> p ch", p=P),
                )
                nc.sync.dma_start(
                    bias_v1, bqkv_d[2 * DSH : 3 * DSH].rearrange("f -> 1 f")
                )
                nc.gpsimd.partition_broadcast(bias_v, bias_v1, channels=P)
            emit_qk(0, pj, qTs, xTs)
            emit_qk(1, pj, qTs, xTs)
        py01s = [py_pair(), py_pair()]
        if R["pending"] is not None:
            emit_norm_g(tc, R, R["pending"])

        # filler units: previous slices' projections + this pair's K (even
        # slice only) and this slice's V
        if R["pending"] is not None:
            p = R["pending"]
            for qq in range(TPS):
                R["proj_fill"].append((p[0], qq, p[4], p[5]))
        R["pending"] = None
        if spar == 0:
            kv_fill = [("qk", 2), ("qk", 3), ("v", 0), ("v", 1)]
        else:
            kv_fill = [("v", 0), ("v", 1)]

        # hp-blocked order: with 2 ps slots this gives depth-2 run-ahead
        # within each head-pair's S->exp chain. History tiles come in fused
        # kt-pairs (2si is always even).
        hist_units = [
            (hp, kt0) for hp in range(2) for kt0 in range(0, TPS * si, 2)
        ]

        def pop_filler(allow_kv=True, allow_proj=True):
            if allow_kv and kv_fill:
                f = kv_fill.pop(0)
                if f[0] == "qk":
                    emit_qk(f[1], pj, qTs, xTs)
                else:
                    emit_v(si, f[1], xTs, xoff)
                return True
            if allow_proj and R["proj_fill"]:
                f_si, qq, yT_ref, wp_ref = R["proj_fill"].pop(0)
                ob_t = outsb_pool.tile([P, C], bf16, name="ob_t")
                emit_proj_g(tc, R, f_si, qq, yT_ref, wp_ref, ob_t, out_d)
                return True
            return False

        for i, (hp, kt0) in enumerate(hist_units):
            emit_s_pair(si, hp, kt0, qTs, xoff, py01s[hp])
            pop_filler()
        # K/V for this slice must be complete before the diagonal
        while pop_filler(allow_proj=False):
            pass
        for hp in range(2):
            emit_s_diag(si, hp, qTs, xoff, py01s[hp])
            pop_filler(allow_kv=False)
        emit_pv(0)
        emit_pv(1)
        R["pending"] = (
            si, slice(si * TS, (si + 1) * TS), py01s[0], py01s[1], yT, wp_sb
        )


_NC_CACHE = {}


def get_program(use_bias=False):
    key = ("nc", use_bias)
    if key not in _NC_CACHE:
        _NC_CACHE[key] = build_program(use_bias=use_bias)
    return _NC_CACHE[key]


def shard_inputs(x, w_qkv, b_qkv, w_proj):
    """Per-core input dicts: core c -> batch c//4, head-group c%4."""
    x = np.asarray(x, dtype=np.float32).astype(NPBF16)
    w_qkv = np.asarray(w_qkv, dtype=np.float32).astype(NPBF16)
    b_qkv = np.asarray(b_qkv, dtype=np.float32)
    w_proj = np.asarray(w_proj, dtype=np.float32).astype(NPBF16)
    in_maps = []
    for c in range(NCORES):
        b, g = divmod(c, NCORES // B)
        cols = []
        for r_ in range(3):  # q, k, v regions
            lo = r_ * C + g * DSH
            cols.append(np.arange(lo, lo + DSH))
        cols = np.concatenate(cols)
        wq = w_qkv[:, cols]  # [C, 3*DSH]
        # chunk-major device layout: [ch, p, cs, fo]
        wq_dev = np.ascontiguousarray(
            wq.reshape(CS, P, 6, P).transpose(2, 1, 0, 3)
        )
        in_maps.append(
            {
                "x": np.ascontiguousarray(
                    x[b].reshape(T, CS, P).transpose(2, 1, 0)
                ),
                "wqkv": wq_dev,
                "bqkv": np.ascontiguousarray(b_qkv[cols]),
                "wproj": np.ascontiguousarray(w_proj[g * DSH : (g + 1) * DSH, :]),
            }
        )
    return in_maps


def kernel(x, w_qkv, b_qkv, w_proj, b_proj, _trace=False):
    use_bias = bool(np.any(np.asarray(b_qkv)))
    nc = get_program(use_bias)
    in_maps = shard_inputs(x, w_qkv, b_qkv, w_proj)
    res = run_bass_kernel_spmd(nc, in_maps, core_ids=list(range(NCORES)), trace=_trace)
    out = np.zeros((B, T, C), dtype=np.float32)
    for c in range(NCORES):
        out[c // (NCORES // B)] += res.results[c]["out"].astype(np.float32)
    out += np.asarray(b_proj, dtype=np.float32)[None, None, :]
    if _trace:
        kernel._last_results = res
    return out


# revision 27
# speedup vs baseline: 1.0637x; 1.0114x over previous
"""Causal self-attention Trainium2 kernel (B=2, T=2048, C=1024, H=16, D=64).

Sharding: 8 cores = data-parallel on B (2) x tensor-parallel on heads (16/4=4
heads per core). Column-parallel Wqkv, row-parallel Wproj; the row-parallel
partial outputs are summed on the host.

v8 design (instruction-count reduction + cross-rep pipelining + fused
diagonal):
  - bf16 datapath, fp32 PSUM accumulation, 256-token attention slices,
    feature-major host-transposed x, flash-style S^T attention with the
    65th-row-of-ones denominator trick and PE row-group (tile_position)
    pairing of the two heads of a 128-partition pair.
  - q/k projections per slice-PAIR with 512-column streams; 512-column
    output projection streams; DVE triangular-mask multiply for the causal
    diagonal; ones/mask initialized once outside the rep loop.
  - all SBUF/PSUM pools and the large persistent tiles live at program
    scope: double-buffered by rep parity (wq/kT/yT/wp/vaug) or ring-shared
    (x, qts, expS, psum pools). Consecutive reps therefore pipeline — the
    next rep's weight/x DMAs and qkv matmuls overlap the previous rep's
    attention tail instead of serializing on pool teardown.
"""

import numpy as np

import concourse.bacc as bacc
import concourse.mybir as mybir
import concourse.tile as tile
from concourse.bass_utils import run_bass_kernel_spmd

B, T, C, H, D = 2, 2048, 1024, 16, 64
NCORES = 8
HPC = H // (NCORES // B)  # 4 heads per core
DSH = HPC * D             # 256 head-dims per core
P = 128
TS = 256                  # q/t slice width (attention granularity)
PW = 512                  # slice-pair width (qkv/proj stream width)
NTS = T // TS             # 8 slices
NT = T // P               # 16 k-tiles
CS = C // P               # 8 contraction subtiles
TPS = TS // P             # 2 t-tiles per slice

f32 = mybir.dt.float32
bf16 = mybir.dt.bfloat16
FP = mybir.ActivationFunctionType
NPBF16 = mybir.dt.np(bf16)


def build_program(reps=1, use_bias=False):
    nc = bacc.Bacc("TRN2", debug=False, num_devices=NCORES)
    x_d = nc.dram_tensor("x", [P, CS, T], bf16, kind="ExternalInput").ap()
    wqkv_d = nc.dram_tensor("wqkv", [6, P, CS, P], bf16, kind="ExternalInput").ap()
    bqkv_d = nc.dram_tensor("bqkv", [3 * DSH], f32, kind="ExternalInput").ap()
    wproj_d = nc.dram_tensor("wproj", [DSH, C], bf16, kind="ExternalInput").ap()
    out_d = nc.dram_tensor("out", [T, C], bf16, kind="ExternalOutput").ap()

    with tile.TileContext(nc) as tc:
        from contextlib import ExitStack

        ctx = ExitStack()
        with ctx:
            ep = ctx.enter_context
            gpool = ep(tc.tile_pool(name="globals", bufs=1))
            R = {}
            # rep-parity double buffers
            R["wq_sb"] = [gpool.tile([P, 6, CS, P], bf16, name=f"wq{i}") for i in range(2)]
            R["kT_sb"] = [gpool.tile([P, 2, T], bf16, name=f"kT{i}") for i in range(2)]
            R["yT"] = [gpool.tile([P, 2, T], bf16, name=f"yT{i}") for i in range(2)]
            R["wp_sb"] = [gpool.tile([P, 2, C], bf16, name=f"wp{i}") for i in range(2)]
            R["vaug"] = [gpool.tile([P, NT, HPC, 65], bf16, name=f"va{i}") for i in range(2)]
            R["mask_sb"] = gpool.tile([P, P], bf16, name="mask")
            R["mask2"] = gpool.tile([P, 2, P], bf16, name="mask2")
            # bias tiles (unused when use_bias=False)
            R["bias_col"] = gpool.tile([P, 4], f32, name="bias_col")
            R["bias_v"] = gpool.tile([P, DSH], f32, name="bias_v")
            R["bias_v1"] = gpool.tile([1, DSH], f32, name="bias_v1")
            # shared pools (ring-rotated across reps)
            R["xsb"] = ep(tc.tile_pool(name="xsb", bufs=4))
            R["qts"] = ep(tc.tile_pool(name="qts", bufs=2))
            R["expS"] = ep(tc.tile_pool(name="expS", bufs=6))
            R["bc"] = ep(tc.tile_pool(name="bc", bufs=6))
            R["outsb"] = ep(tc.tile_pool(name="outsb", bufs=4))
            R["pmm"] = ep(tc.tile_pool(name="pmm", bufs=2, space="PSUM"))
            R["ps"] = ep(tc.tile_pool(name="ps", bufs=2, space="PSUM"))
            R["py"] = ep(tc.tile_pool(name="py", bufs=2, space="PSUM"))

            # norm/proj deferral state carried across reps: the last slice's
            # softmax normalization and output projections of rep n run as
            # PE fillers inside rep n+1 instead of serializing at the tail
            R["pending"] = None
            R["proj_fill"] = []
            for va in R["vaug"]:
                nc.vector.memset(va[:, :, :, 64], 1.0)
            nc.vector.memset(R["mask_sb"], 1.0)
            nc.gpsimd.affine_select(
                out=R["mask_sb"],
                in_=R["mask_sb"],
                compare_op=mybir.AluOpType.is_ge,
                fill=0.0,
                base=0,
                channel_multiplier=-1,
                pattern=[[1, P]],
            )
            nc.vector.memset(R["mask2"], 1.0)
            nc.gpsimd.affine_select(
                out=R["mask2"],
                in_=R["mask2"],
                compare_op=mybir.AluOpType.is_ge,
                fill=0.0,
                base=0,
                channel_multiplier=-1,
                pattern=[[0, 2], [1, P]],
            )
            for rep in range(reps):
                kernel_body(tc, rep, R, x_d, wqkv_d, bqkv_d, wproj_d, out_d,
                            use_bias)
            flush_tail(tc, R, out_d)
    nc.compile()
    return nc


def flush_tail(tc, R, out_d):
    """Emit the final rep's deferred norm + projections."""
    nc = tc.nc
    if R["pending"] is not None:
        emit_norm_g(tc, R, R["pending"])
        f_si = R["pending"][0]
        yT_ref, wp_ref = R["pending"][4], R["pending"][5]
        for qq in range(TPS):
            R["proj_fill"].append((f_si, qq, yT_ref, wp_ref))
        R["pending"] = None
    for f_si, qq, yT_ref, wp_ref in R["proj_fill"]:
        ob_t = R["outsb"].tile([P, C], bf16, name="ob_t")
        emit_proj_g(tc, R, f_si, qq, yT_ref, wp_ref, ob_t, out_d)
    R["proj_fill"] = []


def emit_norm_g(tc, R, p):
    nc = tc.nc
    f_si, f_qsl, f_py0, f_py1, yT_ref, wp_ref = p
    for hp, py_t in ((0, f_py0), (1, f_py1)):
        rc_t = R["bc"].tile([1, 2, TS], f32, name="rc_t", tag="rc")
        nc.vector.reciprocal(rc_t, py_t[64:65, :, :])
        bc_t = R["bc"].tile([64, 2, TS], f32, name="bc_t", tag="bc")
        nc.gpsimd.partition_broadcast(bc_t, rc_t, channels=64)
        for hh in range(2):
            hb = hh * 64
            nc.vector.tensor_mul(
                yT_ref[hb : hb + 64, hp, f_qsl],
                py_t[0:64, hh, :],
                bc_t[:, hh, :],
            )


def emit_proj_g(tc, R, f_si, qq, yT_ref, wp_ref, ob_t, out_d):
    nc = tc.nc
    qt = f_si * TPS + qq
    for cc in range(2):
        po_t = R["pmm"].tile([P, PW], f32, name="po_t", tag="pmm")
        for chp in range(2):
            nc.tensor.matmul(
                po_t,
                lhsT=yT_ref[:, chp, qt * P : (qt + 1) * P],
                rhs=wp_ref[:, chp, cc * PW : (cc + 1) * PW],
                start=(chp == 0),
                stop=(chp == 1),
            )
        nc.vector.tensor_copy(ob_t[:, cc * PW : (cc + 1) * PW], po_t)
    nc.sync.dma_start(out_d[qt * P : (qt + 1) * P, :], ob_t)


def kernel_body(tc, rep, R, x_d, wqkv_d, bqkv_d, wproj_d, out_d,
                use_bias=False):
    nc = tc.nc
    par_ = rep % 2
    wq_sb = R["wq_sb"][par_]
    kT_sb = R["kT_sb"][par_]
    yT = R["yT"][par_]
    wp_sb = R["wp_sb"][par_]
    vaug = R["vaug"][par_]
    mask_sb = R["mask_sb"]
    mask2 = R["mask2"]
    bias_col, bias_v, bias_v1 = R["bias_col"], R["bias_v"], R["bias_v1"]
    xsb_pool, qts_pool = R["xsb"], R["qts"]
    expS_pool, bc_pool, outsb_pool = R["expS"], R["bc"], R["outsb"]
    pmm_pool, ps_pool, py_pool = R["pmm"], R["ps"], R["py"]
    wq_src = wqkv_d.rearrange("ch p cs f -> p ch cs f")

    # paired q/k emission: 512-column streams over a slice-pair's xT
    def emit_qk(ch, pj, qTs, xTs):
        pq = pmm_pool.tile([P, PW], f32, name="pq", tag="pmm")
        for cs in range(CS):
            nc.tensor.matmul(
                pq,
                lhsT=wq_sb[:, ch, cs, :],
                rhs=xTs[:, cs, :],
                start=(cs == 0),
                stop=(cs == CS - 1),
            )
        if ch < 2:
            dst = qTs[:, ch, :]
        else:
            dst = kT_sb[:, ch - 2, pj * PW : (pj + 1) * PW]
        if use_bias:
            nc.vector.tensor_scalar_add(dst, pq, bias_col[:, ch : ch + 1])
        else:
            nc.vector.tensor_copy(dst, pq)

    def emit_v(si, a, xTs, xoff):
        kt = TPS * si + a
        pv = pmm_pool.tile([P, DSH], f32, name="pv", tag="pmm")
        for cs in range(CS):
            nc.tensor.matmul(
                pv,
                lhsT=xTs[:, cs, xoff + a * P : xoff + (a + 1) * P],
                rhs=wq_sb[:, 4:6, cs, :],
                start=(cs == 0),
                stop=(cs == CS - 1),
            )
        dst = vaug[:, kt, :, 0:64]
        src = pv.rearrange("p (h d) -> p h d", h=HPC)
        if use_bias:
            nc.vector.tensor_add(
                dst, src, bias_v.rearrange("p (h d) -> p h d", h=HPC)
            )
        else:
            nc.vector.tensor_copy(dst, src)

    # Deferred PV per head-pair: emit S+exp for a tile (or hist pair), then
    # flush the pending PVs of the previous tile, keeping PE ahead of ACT.
    pend_pv = [None, None]

    def emit_pv(hp):
        if pend_pv[hp] is None:
            return
        si, py_t, entries = pend_pv[hp]
        pend_pv[hp] = None
        n_k = TPS * (si + 1)
        for kt, qoff, rhss in entries:
            for hh in range(2):
                # the two heads share one PSUM bank: only the first matmul
                # of the group clears it, only the last stops it
                nc.tensor.matmul(
                    py_t[:65, hh, qoff:TS],
                    lhsT=vaug[:, kt, 2 * hp + hh, :],
                    rhs=rhss[hh],
                    start=(kt == 0 and hh == 0),
                    stop=(kt == n_k - 1 and hh == 1),
                )

    def emit_s_pair(si, hp, kt0, qTs, qoff0, py01):
        # two full-width history k-tiles fused into one exp instruction
        ps_t = ps_pool.tile([P, 2, 2, TS], f32, name="ps_t", tag="ps")
        ex_t = expS_pool.tile([P, 2, 2, TS], bf16, name="ex_t")
        for par in range(2):
            for hh in range(2):
                hb = hh * 64
                nc.tensor.matmul(
                    ps_t[:, hh, par, :],
                    lhsT=kT_sb[hb : hb + 64, hp, (kt0 + par) * P : (kt0 + par + 1) * P],
                    rhs=qTs[hb : hb + 64, hp, qoff0 : qoff0 + TS],
                    start=True,
                    stop=True,
                    tile_position=(hb, 0),
                )
        nc.scalar.activation(ex_t, ps_t, FP.Exp, scale=0.125)
        emit_pv(hp)
        pend_pv[hp] = (
            si,
            py01,
            [
                (kt0, 0, [ex_t[:, 0, 0, :], ex_t[:, 1, 0, :]]),
                (kt0 + 1, 0, [ex_t[:, 0, 1, :], ex_t[:, 1, 1, :]]),
            ],
        )

    def emit_s_diag(si, hp, qTs, qoff0, py01):
        # the slice's two diagonal k-tiles fused: one exp per head-pair, one
        # two-head mask multiply per k-tile. ps[:, hh, 1, 0:P] is never
        # written; its exp output is masked garbage that no PV reads.
        kt0 = TPS * si
        ps_t = ps_pool.tile([P, 2, 2, TS], f32, name="ps_t", tag="ps")
        ex_t = expS_pool.tile([P, 2, 2, TS], bf16, name="ex_t")
        for kd in range(2):
            qoff = kd * P
            for hh in range(2):
                hb = hh * 64
                nc.tensor.matmul(
                    ps_t[:, hh, kd, qoff:TS],
                    lhsT=kT_sb[hb : hb + 64, hp, (kt0 + kd) * P : (kt0 + kd + 1) * P],
                    rhs=qTs[hb : hb + 64, hp, qoff0 + qoff : qoff0 + TS],
                    start=True,
                    stop=True,
                    tile_position=(hb, 0),
                )
        nc.scalar.activation(ex_t, ps_t, FP.Exp, scale=0.125)
        nc.vector.tensor_mul(ex_t[:, :, 0, 0:P], ex_t[:, :, 0, 0:P], mask2)
        nc.vector.tensor_mul(ex_t[:, :, 1, P:TS], ex_t[:, :, 1, P:TS], mask2)
        emit_pv(hp)
        pend_pv[hp] = (
            si,
            py01,
            [
                (kt0, 0, [ex_t[:, 0, 0, :], ex_t[:, 1, 0, :]]),
                (kt0 + 1, P, [ex_t[:, 0, 1, P:TS], ex_t[:, 1, 1, P:TS]]),
            ],
        )

    def py_pair():
        return py_pool.tile([P, 2, TS], f32, name="py", tag="py")

    xtiles = {}

    def x_load(pj, split=False):
        t_ = xsb_pool.tile([P, CS, PW], bf16, name="x_sb")
        if split:
            h = CS // 2
            nc.sync.dma_start(t_[:, 0:h], x_d[:, 0:h, pj * PW : (pj + 1) * PW])
            nc.sync.dma_start(t_[:, h:CS], x_d[:, h:CS, pj * PW : (pj + 1) * PW])
        else:
            nc.sync.dma_start(t_, x_d[:, :, pj * PW : (pj + 1) * PW])
        xtiles[pj] = t_

    nc.sync.dma_start(wq_sb[:, 0], wq_src[:, 0])
    x_load(0, split=True)
    for ch in range(1, 4):
        nc.sync.dma_start(wq_sb[:, ch], wq_src[:, ch])
    nc.sync.dma_start(wq_sb[:, 4:6], wq_src[:, 4:6])
    x_load(1)
    nc.sync.dma_start(wp_sb, wproj_d.rearrange("(ch p) f -> p ch f", p=P))
    for pj in range(2, NTS // 2):
        x_load(pj)
    qTs = None
    for si in range(NTS):
        pj, spar = divmod(si, 2)
        xTs = xtiles[pj] if spar == 0 else xtiles.pop(pj)
        xoff = spar * TS
        if spar == 0:
            qTs = qts_pool.tile([P, 2, PW], bf16, name="qTs")
            if si == 0 and use_bias:
                nc.sync.dma_start(
                    bias_col,
                    bqkv_d[0 : 4 * P].rearrange("(ch p) -> p ch", p=P),
                )
                nc.sync.dma_start(
                    bias_v1, bqkv_d[2 * DSH : 3 * DSH].rearrange("f -> 1 f")
                )
                nc.gpsimd.partition_broadcast(bias_v, bias_v1, channels=P)
            emit_qk(0, pj, qTs, xTs)
            emit_qk(1, pj, qTs, xTs)
        py01s = [py_pair(), py_pair()]
        if R["pending"] is not None:
            emit_norm_g(tc, R, R["pending"])

        # filler units: previous slices' projections + this pair's K (even
        # slice only) and this slice's V
        if R["pending"] is not None:
            p = R["pending"]
            for qq in range(TPS):
                R["proj_fill"].append((p[0], qq, p[4], p[5]))
        R["pending"] = None
        if spar == 0:
            kv_fill = [("qk", 2), ("qk", 3), ("v", 0), ("v", 1)]
        else:
            kv_fill = [("v", 0), ("v", 1)]

        # hp-blocked order: with 2 ps slots this gives depth-2 run-ahead
        # within each head-pair's S->exp chain. History tiles come in fused
        # kt-pairs (2si is always even).
        hist_units = [
            (hp, kt0) for hp in range(2) for kt0 in range(0, TPS * si, 2)
        ]

        def pop_filler(allow_kv=True, allow_proj=True):
            if allow_kv and kv_fill:
                f = kv_fill.pop(0)
                if f[0] == "qk":
                    emit_qk(f[1], pj, qTs, xTs)
                else:
                    emit_v(si, f[1], xTs, xoff)
                return True
            if allow_proj and R["proj_fill"]:
                f_si, qq, yT_ref, wp_ref = R["proj_fill"].pop(0)
                ob_t = outsb_pool.tile([P, C], bf16, name="ob_t")
                emit_proj_g(tc, R, f_si, qq, yT_ref, wp_ref, ob_t, out_d)
                return True
            return False

        for i, (hp, kt0) in enumerate(hist_units):
            emit_s_pair(si, hp, kt0, qTs, xoff, py01s[hp])
            pop_filler()
        # K/V for this slice must be complete before the diagonal
        while pop_filler(allow_proj=False):
            pass
        for hp in range(2):
            emit_s_diag(si, hp, qTs, xoff, py01s[hp])
            pop_filler(allow_kv=False)
        emit_pv(0)
        emit_pv(1)
        R["pending"] = (
            si, slice(si * TS, (si + 1) * TS), py01s[0], py01s[1], yT, wp_sb
        )


_NC_CACHE = {}


def get_program(use_bias=False):
    key = ("nc", use_bias)
    if key not in _NC_CACHE:
        _NC_CACHE[key] = build_program(use_bias=use_bias)
    return _NC_CACHE[key]


def shard_inputs(x, w_qkv, b_qkv, w_proj):
    """Per-core input dicts: core c -> batch c//4, head-group c%4."""
    x = np.asarray(x, dtype=np.float32).astype(NPBF16)
    w_qkv = np.asarray(w_qkv, dtype=np.float32).astype(NPBF16)
    b_qkv = np.asarray(b_qkv, dtype=np.float32)
    w_proj = np.asarray(w_proj, dtype=np.float32).astype(NPBF16)
    in_maps = []
    for c in range(NCORES):
        b, g = divmod(c, NCORES // B)
        cols = []
        for r_ in range(3):  # q, k, v regions
            lo = r_ * C + g * DSH
            cols.append(np.arange(lo, lo + DSH))
        cols = np.concatenate(cols)
        wq = w_qkv[:, cols]  # [C, 3*DSH]
        # chunk-major device layout: [ch, p, cs, fo]
        wq_dev = np.ascontiguousarray(
            wq.reshape(CS, P, 6, P).transpose(2, 1, 0, 3)
        )
        in_maps.append(
            {
                "x": np.ascontiguousarray(
                    x[b].reshape(T, CS, P).transpose(2, 1, 0)
                ),
                "wqkv": wq_dev,
                "bqkv": np.ascontiguousarray(b_qkv[cols]),
                "wproj": np.ascontiguousarray(w_proj[g * DSH : (g + 1) * DSH, :]),
            }
        )
    return in_maps


def kernel(x, w_qkv, b_qkv, w_proj, b_proj, _trace=False):
    use_bias = bool(np.any(np.asarray(b_qkv)))
    nc = get_program(use_bias)
    in_maps = shard_inputs(x, w_qkv, b_qkv, w_proj)
    res = run_bass_kernel_spmd(nc, in_maps, core_ids=list(range(NCORES)), trace=_trace)
    out = np.zeros((B, T, C), dtype=np.float32)
    for c in range(NCORES):
        out[c // (NCORES // B)] += res.results[c]["out"].astype(np.float32)
    out += np.asarray(b_proj, dtype=np.float32)[None, None, :]
    if _trace:
        kernel._last_results = res
    return out
